# revision 20
# baseline (speedup 1.0000x reference)
"""MoE positionwise FFN (top-2 of 8 experts) on 8 TRN2 NeuronCores.

Strategy: pair-wise expert-parallel with an F-split, host-routed.
The router (logits -> top-2 -> softmax gates) is exact fp32 on host,
as is the final partial-sum + gated scatter-add combine. Experts are
paired (heavy with light); core pair (2p, 2p+1) jointly owns experts
(A_p, B_p): each core holds HALF of F for BOTH experts and processes
ALL of the pair's tokens, producing partial y (summed on host). This
balances compute across cores: per-core work = (segA + segB)/2 token
equivalents where segA/segB are the global max heavy/light expert
counts — less than the single max count an unsplit expert-parallel
layout is stuck with. b2 is added only by the q=0 core of each pair
(the q=1 core gets a zero bias), so the host-side sum is exact.

Per segment the FFN is h = relu(w1_half @ x + b1_half);
y_part = w2_half @ h (+ b2), weight-stationary bf16 matmuls with fp32
PSUM accumulation, over token blocks sized <= 512 (PSUM bank limit).

All device inputs are pre-permuted on host into the exact SBUF layout
so every DMA is a contiguous slice (>=8KB per-partition runs, full
HBM bandwidth). All big transfers ride the SP (sync) HWDGE ring,
depth-3 pipelined so the ring streams back-to-back with completion
receipts overlapped; the ACT engine carries only tiny dep-free bias
DMAs (stream DMAs there would head-of-line-block the activations and
stall PSUM drain). The matmul lhsT reads the weights with a small
free-dim stride, which LDWEIGHTS tolerates (hidden behind matmuls).
The PE is pre-warmed so the HAM clock gate is at 8/8 when the real
matmuls start.

Self-contained: hardcodes shapes for B=2,T=2048,D=1024,F=4096,E=8,K=2.
"""
import math

import numpy as np
import ml_dtypes

S = 4096
D = 1024
F = 4096
E = 8
FH = F // 2       # per-core F half
NTD = D // 128    # 8 d-tiles
NTFH = FH // 128  # 16 f-tiles per half

_cache: dict = {}
LAST_RES = None


def _plan_seg(maxcnt: int):
    C = max(64, int(math.ceil(maxcnt / 16)) * 16)
    if C <= 512:
        return (C,)
    if C <= 1024:
        s0 = int(math.ceil(C / 2 / 16)) * 16
        return (s0, C - s0)
    blocks = []
    rem = C
    while rem > 896:
        blocks.append(448)
        rem -= 448
    if rem > 448:
        s1 = int(math.ceil(rem / 2 / 16)) * 16
        blocks.extend([s1, rem - s1])
    elif rem > 0:
        blocks.append(rem)
    return tuple(blocks)


def _build(plans: tuple):
    """plans = (segA_blocks, segB_blocks)."""
    import concourse.bacc as bacc
    import concourse.tile as tile
    import concourse.mybir as mybir
    from concourse.tile import add_dep_helper

    f32 = mybir.dt.float32
    bf16 = mybir.dt.bfloat16
    ACT = mybir.ActivationFunctionType

    nc = bacc.Bacc("TRN2", target_bir_lowering=False, debug=False, num_devices=8)

    segs = []
    for sname, blocks in zip(("a", "b"), plans):
        segs.append({
            "blocks": blocks,
            "x_d": [nc.dram_tensor(f"x{sname}{i}", [128, NTD, tb], bf16,
                                   kind="ExternalInput")
                    for i, tb in enumerate(blocks)],
            "w1_d": nc.dram_tensor(f"w1{sname}", [128, FH, NTD], bf16,
                                   kind="ExternalInput"),
            "w2_d": nc.dram_tensor(f"w2{sname}", [128, D, NTFH], bf16,
                                   kind="ExternalInput"),
            "b1_d": nc.dram_tensor(f"b1{sname}", [128, NTFH], f32,
                                   kind="ExternalInput"),
            "b2_d": nc.dram_tensor(f"b2{sname}", [128, NTD], f32,
                                   kind="ExternalInput"),
            "y_d": [nc.dram_tensor(f"y{sname}{i}", [128, NTD, tb], bf16,
                                   kind="ExternalOutput")
                    for i, tb in enumerate(blocks)],
        })
    maxtb = max(max(s["blocks"]) for s in segs)

    with tile.TileContext(nc) as tc:
        with (
            tc.tile_pool(name="wpool", bufs=1) as wpool,
            tc.tile_pool(name="xr", bufs=1) as xr,
            tc.tile_pool(name="small", bufs=1) as small,
            tc.tile_pool(name="hpool", bufs=1) as hpool,
            tc.tile_pool(name="ypool", bufs=2) as ypool,
            tc.tile_pool(name="psH", bufs=4, space="PSUM") as psH,
            tc.tile_pool(name="psY", bufs=3, space="PSUM") as psY,
            tc.tile_pool(name="psW", bufs=1, space="PSUM") as psW,
        ):
            # ---- PE pre-warm: trip the HAM activity window during the
            # initial DMA so real matmuls start at 2.4 GHz.
            junk = small.tile([128, 256], bf16)
            nc.vector.memset(junk[:], 0.0)
            wps = psW.tile([128, 256], f32)
            for _ in range(26):
                nc.tensor.matmul(wps[:], lhsT=junk[:, 0:128], rhs=junk[:],
                                 start=True, stop=True)

            # ---- DMA streams: SP ring, depth-3 pipelined.
            ring = []

            def put(fn):
                d = fn()
                if ring:
                    add_dep_helper(d.ins, ring[-1].ins, sync=False,
                                   reason="ring order")
                if len(ring) >= 4:
                    add_dep_helper(d.ins, ring[-4].ins, sync=True,
                                   reason="ring depth-4")
                ring.append(d)
                return d

            for si, s in enumerate(segs):
                # seg-B biases are needed only ~130us in: load them via the
                # GPSIMD SWDGE queue so they don't hold HWDGE sem lanes
                # during the startup stream.
                deng = nc.scalar if si == 0 else nc.gpsimd
                b1_sb = small.tile([128, NTFH], f32, name=f"b1sb{id(s)%97}")
                deng.dma_start(out=b1_sb[:], in_=s["b1_d"][:, :])
                b2_sb = small.tile([128, NTD], f32, name=f"b2sb{id(s)%97}")
                deng.dma_start(out=b2_sb[:], in_=s["b2_d"][:, :])
                s["b1_sb"], s["b2_sb"] = b1_sb, b2_sb
                s["xg"] = [xr.tile([128, NTD, tb], bf16, tag=f"xg{id(s)%97}_{i}",
                                   name=f"xg{id(s)%97}_{i}")
                           for i, tb in enumerate(s["blocks"])]
                s["w1_sb"] = wpool.tile([128, FH, NTD], bf16,
                                        name=f"w1sb{id(s)%97}")
                s["w2_sb"] = wpool.tile([128, D, NTFH], bf16,
                                        name=f"w2sb{id(s)%97}")

            sa, sb = segs
            # seg A: x block0 (dt halves), then w1a chunks (fine first),
            # w2a chunks, remaining xa blocks.
            put(lambda: nc.sync.dma_start(out=sa["xg"][0][:, 0:2, :],
                                          in_=sa["x_d"][0][:, 0:2, :]))
            put(lambda s=sa: nc.sync.dma_start(
                out=s["w1_sb"][:, 0:128, :], in_=s["w1_d"][:, 0:128, :]))
            for dl in (2, 4, 6):
                put(lambda dl=dl: nc.sync.dma_start(
                    out=sa["xg"][0][:, dl : dl + 2, :],
                    in_=sa["x_d"][0][:, dl : dl + 2, :]))
            w1cuts = [128, 256, 512, 1024, 1536, FH]
            for lo, hi in zip(w1cuts, w1cuts[1:]):
                put(lambda s=sa, lo=lo, hi=hi: nc.sync.dma_start(
                    out=s["w1_sb"][:, lo:hi, :], in_=s["w1_d"][:, lo:hi, :]))
            DC = D // 4
            for i in range(4):
                put(lambda s=sa, i=i: nc.sync.dma_start(
                    out=s["w2_sb"][:, i * DC : (i + 1) * DC, :],
                    in_=s["w2_d"][:, i * DC : (i + 1) * DC, :]))
            for i in range(1, len(sa["blocks"])):
                put(lambda s=sa, i=i: nc.sync.dma_start(
                    out=s["xg"][i][:], in_=s["x_d"][i][:, :, :]))
            # seg B weights + x (consumed later; coarser chunks fine)
            for i in range(4):
                put(lambda s=sb, i=i: nc.sync.dma_start(
                    out=s["w1_sb"][:, i * (FH // 4) : (i + 1) * (FH // 4), :],
                    in_=s["w1_d"][:, i * (FH // 4) : (i + 1) * (FH // 4), :]))
            for i in range(4):
                put(lambda s=sb, i=i: nc.sync.dma_start(
                    out=s["w2_sb"][:, i * DC : (i + 1) * DC, :],
                    in_=s["w2_d"][:, i * DC : (i + 1) * DC, :]))
            for i in range(len(sb["blocks"])):
                put(lambda s=sb, i=i: nc.sync.dma_start(
                    out=s["xg"][i][:], in_=s["x_d"][i][:, :, :]))

            # ---- FFN over segments and token blocks --------------------
            for s in segs:
                for blk, TB in enumerate(s["blocks"]):
                    xg_blk = s["xg"][blk]
                    h_sb = hpool.tile([128, NTFH, maxtb], bf16, tag="h")
                    for ft in range(NTFH):
                        hp = psH.tile([128, TB], f32, tag="hps")
                        for dt in range(NTD):
                            nc.tensor.matmul(
                                hp[:],
                                lhsT=s["w1_sb"][:, ft * 128 : (ft + 1) * 128, dt],
                                rhs=xg_blk[:, dt, :],
                                start=(dt == 0),
                                stop=(dt == NTD - 1),
                            )
                        nc.scalar.activation(out=h_sb[:, ft, 0:TB], in_=hp[:],
                                             func=ACT.Relu,
                                             bias=s["b1_sb"][:, ft : ft + 1],
                                             scale=1.0)
                    y_blk = ypool.tile([128, NTD, TB], bf16, tag="y")
                    lastblk = (s is segs[-1]) and blk == len(s["blocks"]) - 1
                    for dt in range(NTD):
                        halves = ([(0, TB)] if not (lastblk and dt == NTD - 1)
                                  else [(0, TB // 2), (TB // 2, TB)])
                        for lo, hi in halves:
                            yp = psY.tile([128, hi - lo], f32, tag="yps")
                            for ft in range(NTFH):
                                nc.tensor.matmul(
                                    yp[:],
                                    lhsT=s["w2_sb"][:, dt * 128 : (dt + 1) * 128, ft],
                                    rhs=h_sb[:, ft, lo:hi],
                                    start=(ft == 0),
                                    stop=(ft == NTFH - 1),
                                )
                            nc.vector.tensor_scalar_add(y_blk[:, dt, lo:hi], yp[:],
                                                        s["b2_sb"][:, dt : dt + 1])
                            if lastblk:
                                put(lambda s=s, blk=blk, dt=dt, y_blk=y_blk,
                                    lo=lo, hi=hi:
                                    nc.sync.dma_start(out=s["y_d"][blk][:, dt, lo:hi],
                                                      in_=y_blk[:, dt, lo:hi]))
                    if not ((s is segs[-1]) and blk == len(s["blocks"]) - 1):
                        put(lambda s=s, blk=blk, y_blk=y_blk: nc.sync.dma_start(
                            out=s["y_d"][blk][:, :, :], in_=y_blk[:]))

    nc.compile()
    return nc


def _get_nc(plans: tuple):
    if plans not in _cache:
        _cache[plans] = _build(plans)
    return _cache[plans]


def kernel(x, gate_w, w1, b1, w2, b2, k):
    from concourse.bass_utils import run_bass_kernel_spmd

    assert int(k) == 2
    x = np.asarray(x, dtype=np.float32)
    gate_w = np.asarray(gate_w, dtype=np.float32)
    w1 = np.asarray(w1, dtype=np.float32)
    b1 = np.asarray(b1, dtype=np.float32)
    w2 = np.asarray(w2, dtype=np.float32)
    b2 = np.asarray(b2, dtype=np.float32)
    B, T, _ = x.shape
    xf = x.reshape(S, D)

    # Router (exact fp32, matching the reference's top-2 renormalized
    # softmax; gates applied host-side during the merge).
    logits = xf @ gate_w.T
    top2 = np.argpartition(-logits, 2, axis=1)[:, :2]
    topv = np.take_along_axis(logits, top2, axis=1)
    ex = np.exp(topv - topv.max(axis=1, keepdims=True))
    gsm = ex / ex.sum(axis=1, keepdims=True)
    gates = np.zeros((S, E), dtype=np.float32)
    np.put_along_axis(gates, top2, gsm.astype(np.float32), axis=1)

    sel = np.zeros((S, E), dtype=bool)
    np.put_along_axis(sel, top2, True, axis=1)
    toks = [np.nonzero(sel[:, e])[0] for e in range(E)]
    cnts = np.array([len(t) for t in toks])

    # pair heavy experts with light ones: the 4 heaviest are segment-A
    # (first) experts, the 4 lightest segment-B; capacities are the
    # global maxima so one SPMD program serves all pairs.
    order = np.argsort(-cnts, kind="stable")
    firsts, seconds = order[:4], order[4:]
    pairs = list(zip(firsts.tolist(), seconds.tolist()))
    segA = _plan_seg(int(cnts[firsts].max()))
    segB = _plan_seg(int(cnts[seconds].max()))
    plans = (segA, segB)
    offsA = [sum(segA[:i]) for i in range(len(segA))]
    offsB = [sum(segB[:i]) for i in range(len(segB))]

    nc = _get_nc(plans)

    xfT16 = np.ascontiguousarray(xf.T).astype(ml_dtypes.bfloat16)  # [D, S]

    def xblocks(toklist, blocks, offs, prefix, im):
        C = sum(blocks)
        tp = np.zeros(C, dtype=np.int64)
        tp[: len(toklist)] = toklist
        for i, tb in enumerate(blocks):
            g = xfT16[:, tp[offs[i] : offs[i] + tb]]          # [D, tb]
            im[f"{prefix}{i}"] = np.ascontiguousarray(
                g.reshape(NTD, 128, tb).transpose(1, 0, 2))

    in_maps = []
    for c in range(E):
        p, q = divmod(c, 2)
        A, Bx = pairs[p]
        fsl = slice(q * FH, (q + 1) * FH)
        im = {}
        for sname, ee in (("a", A), ("b", Bx)):
            im[f"w1{sname}"] = np.ascontiguousarray(
                w1[ee][fsl, :].astype(ml_dtypes.bfloat16)
                .reshape(FH, NTD, 128).transpose(2, 0, 1))
            im[f"w2{sname}"] = np.ascontiguousarray(
                w2[ee][:, fsl].astype(ml_dtypes.bfloat16)
                .reshape(D, NTFH, 128).transpose(2, 0, 1))
            im[f"b1{sname}"] = np.ascontiguousarray(
                b1[ee][fsl].reshape(NTFH, 128).T)
            im[f"b2{sname}"] = (np.ascontiguousarray(b2[ee].reshape(NTD, 128).T)
                                if q == 0 else
                                np.zeros((128, NTD), dtype=np.float32))
        xblocks(toks[A], segA, offsA, "xa", im)
        xblocks(toks[Bx], segB, offsB, "xb", im)
        in_maps.append(im)

    res = run_bass_kernel_spmd(nc, in_maps, core_ids=list(range(8)))
    global LAST_RES
    LAST_RES = res

    out = np.zeros((S, D), dtype=np.float32)
    for p in range(4):
        A, Bx = pairs[p]
        for sname, ee, blocks in (("a", A, segA), ("b", Bx, segB)):
            cnt = len(toks[ee])
            yt = None
            for q in (0, 1):
                ycat = np.concatenate(
                    [np.asarray(res.results[2 * p + q][f"y{sname}{i}"])
                     .astype(np.float32).transpose(1, 0, 2).reshape(D, tb)
                     for i, tb in enumerate(blocks)], axis=1)   # [D, C]
                yt = ycat if yt is None else yt + ycat
            out[toks[ee]] += yt[:, :cnt].T * gates[toks[ee], ee][:, None]
    return out.reshape(B, T, D)
